# revision 33
# baseline (speedup 1.0000x reference)
"""Trainium2 Bass kernel: 3x3 VALID conv2d, stride 1.

Full input [32, 64, 112, 112] f32 + weights [128, 64, 3, 3] f32
-> output [32, 128, 110, 110] f32.

Data-parallel across 8 NeuronCores: 4 images per core.

Per-core formulation: conv as PE matmuls, out = lhsT.T @ rhs with
K (contraction, partitions) = 128 = 64 channels x 2 shifted copies,
M (out partitions) = 128 output channels,
N (moving free dim) = up to 4 input-width rows = 448 (<= 512, one PSUM
bank). The 2 rightmost columns of each 112-wide row are conv garbage;
the PSUM->SBUF copy compacts to the valid 110 columns.

Tap coverage per chunk, two schemes:
  T tile (all chunks): partitions 0-63 = image rows 0..111 (A),
          64-127 = rows 1..111 (B).  Matmuls m=0..2 at column offset
          kx apply tap pairs (0,kx)+(1,kx).
  5-MM scheme (chunks with y0 < Y5=64): U tile: partitions 0-63 =
          rows 2..68 (C), 64-127 = same shifted one column.  m=3
          applies (2,0)+(2,1) in one full-K matmul; m=4 applies (2,2)
          on the hi half only -- 5 matmuls for 9 taps.
  6-MM scheme (y0 >= 64): ky=2 taps via T at row offset +1 with
          zero weights on the A half (planes 5-7) -- 6 matmuls, no U.

U is built on-device by two same-partition contiguous SBUF->SBUF DMAs
per band (lo: A shifted +224 elements; hi: B shifted +113), so HBM
input traffic stays at the single-copy ~12.9 MB/core.  Measured mover
rates force the hybrid coverage: an HWDGE ring moves ~110-125 GB/s
serialized, and every extra consumer slows the others, so U is sized
to what the otherwise-idle scalar ring can sustain alone (~57% of
rows), with image 0's hi half on sync's idle early window.  Each queue
has exactly one role -- tensor: matmuls; vector: PSUM casts; scalar:
U builds; gpsimd: T loads; sync: weights + first bands + output -- so
an unfired semaphore never head-blocks latency-critical work.
(Also measured and rejected: streaming U from HBM saturates the
~358 GB/s HBM interface; full-coverage U overloads the rings and
starves the PE; compute-engine tensor_copy builds crawl at 14-25
G elem/s.)

Moving-N per tap is trimmed (n, n-1, n-2, ...) so no rhs read spills
past input row y0+3 (y0+4 for the 6-MM tail rows): only garbage output
columns lose taps.

Inputs are cast to fp16 on the host (fp32 PE is 4x slower; fp32 PSUM
accumulation keeps rel err ~4e-4).  Output is stored fp16 and cast
back to fp32 on the host, halving output HBM traffic.

A short burst of dummy matmuls on a memset tile runs during the DMA
startup window so the PE HAM clock gate flips to 2.4 GHz by the time
real work arrives.

Schedule: chunks are processed in groups of 8 across the 8 PSUM banks,
weight-plane-major (m outer), so consecutive matmuls hit different
banks (drain overlaps fill).
"""

import numpy as np

B_FULL = 32
N_CORES = 8
B_CORE = B_FULL // N_CORES  # 4 images per core
C_IN = 64
C_OUT = 128
H = W = 112
OH = OW = 110
TLEN = 112 * W  # T plane: rows 0..111 (A) / 1..111 + zero pad (B)
Y5B0 = 64  # image 0: 5-MM for y0 >= 64 (runs 6-MM while input streams)
Y5B = 32  # images 1-3: 5-MM for y0 < 32 (sized to scalar-ring bandwidth)
UROWS = Y5B  # images 1-3: U plane rows 2..33 (last 5-MM chunk reads row 33)
ULEN = UROWS * W
U0BASE = Y5B0  # image 0's U plane starts at flat row 64 (rows 66..111)
U0LEN = (OH - Y5B0) * W  # 46 rows

_NC = None


def _img_chunks():
    # per image: 27 chunks of 4 output rows + 1 of 2 rows = 110
    rows_list = [4] * 27 + [2]
    out = []
    y0 = 0
    for r in rows_list:
        out.append((y0, r))
        y0 += r
    assert y0 == OH
    return out


def _build():
    from contextlib import ExitStack

    import concourse.tile as tile
    from concourse import bacc, mybir

    nc = bacc.Bacc("TRN2", target_bir_lowering=False, debug=False)
    x = nc.dram_tensor(
        "x", [B_CORE, 128, TLEN], mybir.dt.float16, kind="ExternalInput"
    )
    w = nc.dram_tensor("w", [128, 8, 128], mybir.dt.float16, kind="ExternalInput")
    y = nc.dram_tensor(
        "y", [B_CORE, C_OUT, OH * OW], mybir.dt.float16, kind="ExternalOutput"
    )

    chunks = [(b, y0, r) for b in range(B_CORE) for (y0, r) in _img_chunks()]
    assert len(chunks) % 8 == 0
    n_groups = len(chunks) // 8

    with tile.TileContext(nc) as tc, ExitStack() as ctx:
        tpool = ctx.enter_context(tc.tile_pool(name="tp", bufs=B_CORE))
        upool0 = ctx.enter_context(tc.tile_pool(name="up0", bufs=1))
        upool = ctx.enter_context(tc.tile_pool(name="up", bufs=B_CORE - 1))
        wpool = ctx.enter_context(tc.tile_pool(name="wp", bufs=1))
        spool = ctx.enter_context(tc.tile_pool(name="sp", bufs=1))
        opool = ctx.enter_context(tc.tile_pool(name="op", bufs=12))
        ppool = ctx.enter_context(tc.tile_pool(name="pp", bufs=8, space="PSUM"))

        wt = wpool.tile([128, 8 * 128], mybir.dt.float16)
        nc.sync.dma_start(wt[:], w.ap().rearrange("p a b -> p (a b)"))

        # PE warmup: HAM clock gate flips to 2.4 GHz after ~3.4us of
        # sustained activity; burn that in while the first x bands load.
        wu = spool.tile([128, 448], mybir.dt.float16)
        nc.vector.memset(wu[:], 0)
        wu_p = ppool.tile([128, 448], mybir.dt.float32, name="wu_p", tag="pt")
        # wide warmup matmuls: high MAC duty cycle is what flips the HAM
        # clock gate (N=64 warmups never flipped it -- LDW time is idle)
        for _ in range(12):
            nc.tensor.matmul(
                wu_p[0:64, 0:448], wu[:, 0:64], wu[:],
                start=True, stop=True, skip_group_check=True,
            )

        xa = x.ap()
        ya = y.ap()

        # Banded loads so the first chunks start early.  Image 0's first
        # two T bands ride the sync queue (earliest to start); the bulk
        # of T streams on gpsimd/SWDGE.  U band k reads only T band k
        # (U edges = T edges - 2).
        t_tiles = [
            tpool.tile([128, TLEN], mybir.dt.float16, name=f"t{b}", tag="t")
            for b in range(B_CORE)
        ]
        u_tiles = [upool0.tile([128, U0LEN], mybir.dt.float16, name="u0", tag="u0")]
        u_tiles += [
            upool.tile([128, ULEN], mybir.dt.float16, name=f"u{b}", tag="u")
            for b in range(1, B_CORE)
        ]
        ubase = [U0BASE, 0, 0, 0]
        # all T bands ride gpsimd/SWDGE -- the only ring measured at
        # ~200+ GB/s for HBM loads (sync's HWDGE ring crawls at ~80)
        TBs = [[0, 6, 16, 26, 34, 49, 64, 89, 112]] + [[0, 16, 34, 61, 89, 112]] * 3
        UBs = [[U0BASE, 87, OH]] + [[0, 14, UROWS]] * 3
        assert UROWS == 32

        def t_issue(b, k):
            lo, hi = TBs[b][k], TBs[b][k + 1]
            nc.gpsimd.dma_start(
                t_tiles[b][:, lo * W : hi * W], xa[b][:, lo * W : hi * W]
            )

        def u_build(b, k, eng=None):
            eng = eng or nc.scalar
            lo, hi = UBs[b][k], UBs[b][k + 1]
            d0, d1 = (lo - ubase[b]) * W, (hi - ubase[b]) * W
            eng.dma_start(
                u_tiles[b][0:64, d0:d1],
                t_tiles[b][0:64, (lo + 2) * W : (hi + 2) * W],
            )
            eng.dma_start(
                u_tiles[b][64:128, d0:d1],
                t_tiles[b][64:128, lo * W + 113 : hi * W + 113],
            )

        # image 0 first, then every image's FIRST band, then the rest:
        # each image's U-build chain (T land -> sem -> scalar ring ->
        # sem, ~6-9us) gets a 10+us head start this way
        for k in range(len(TBs[0]) - 1):
            t_issue(0, k)
        for b in (1, 2, 3):
            t_issue(b, 0)
        for b in (1, 2, 3):
            for k in range(1, len(TBs[b]) - 1):
                t_issue(b, k)
        # scalar's queue carries only U builds, so the in-order semaphore
        # waits (T band b,k) head-block nothing; emission order matches
        # T landing order for a natural pipeline
        # image 0's later U band rides sync (needed last among image 0's
        # bands; frees the scalar ring ~7us earlier for images 1-3)
        u_build(0, 0)
        u_build(0, 1, eng=nc.sync)
        for b in (1, 2, 3):
            for k in range(len(UBs[b]) - 1):
                u_build(b, k)

        def chunk_taps(b, y0, rows):
            n = rows * W
            t, u = t_tiles[b], u_tiles[b]
            # image 0 runs 6-MM while its input streams in and 5-MM on
            # its tail rows; images 1-3 (whose T arrives with slack) run
            # 5-MM on their first rows and 6-MM on the tail
            five = (y0 >= Y5B0) if b == 0 else (y0 < Y5B)
            if five:
                uo = (y0 - ubase[b]) * W
                return [
                    (0, t, y0 * W, n),
                    (1, t, y0 * W + 1, n - 1),
                    (2, t, y0 * W + 2, n - 2),
                    (3, u, uo, n - 1),
                    (4, u, uo + 1, n - 2),
                ]
            return [  # 6-matmul scheme, ky=2 via T at +1 row
                (0, t, y0 * W, n),
                (1, t, y0 * W + 1, n - 1),
                (2, t, y0 * W + 2, n - 2),
                (5, t, (y0 + 1) * W, n),
                (6, t, (y0 + 1) * W + 1, n - 1),
                (7, t, (y0 + 1) * W + 2, n - 2),
            ]

        for g in range(n_groups):
            gchunks = chunks[g * 8 : (g + 1) * 8]
            pts = [
                ppool.tile([128, 448], mybir.dt.float32, name="pt", tag="pt")
                for _ in range(8)
            ]
            taps = [chunk_taps(*c) for c in gchunks]
            for m in range(8):
                for j in range(8):
                    for mi, (mm, src, off, nmv) in enumerate(taps[j]):
                        if mm != m:
                            continue
                        nc.tensor.matmul(
                            pts[j][:, 0:nmv],
                            wt[:, m * 128 : (m + 1) * 128],
                            src[:, off : off + nmv],
                            start=(mi == 0),
                            stop=(mi == len(taps[j]) - 1),
                            skip_group_check=True,
                        )
            # compact + store per 2 chunks: copies start draining PSUM as
            # soon as each pair of banks stops; out DMAs alternate between
            # the sync and scalar rings to balance ring load
            for h in range(4):
                pair = gchunks[2 * h : 2 * h + 2]
                total_rows = sum(r for _, _, r in pair)
                ot = opool.tile([128, 8 * OW], mybir.dt.float16, tag="ot")
                off = 0
                for jj, (b, y0, rows) in enumerate(pair):
                    j = 2 * h + jj
                    psrc = pts[j][:].rearrange("p (r c) -> p r c", c=W)[
                        :, 0:rows, 0:OW
                    ]
                    odst = ot[:, off : off + rows * OW].rearrange(
                        "p (r c) -> p r c", c=OW
                    )
                    # all casts on vector: it issues no DMAs, so PSUM
                    # drain is never head-blocked by an unfired T-band
                    # semaphore on a DMA-issuing queue
                    nc.vector.tensor_copy(odst, psrc)
                    off += rows * OW
                b0, y00, _ = pair[0]
                assert all(b == b0 for b, _, _ in pair)
                # later outputs ride the gpsimd ring (the fast SWDGE ring,
                # idle once T has streamed) to kill the output lag/tail
                oeng = nc.gpsimd if g >= 7 else nc.sync
                oeng.dma_start(
                    ya[b0][:, y00 * OW : y00 * OW + total_rows * OW],
                    ot[:, 0 : total_rows * OW],
                )

    nc.compile()
    return nc


def _get_nc():
    global _NC
    if _NC is None:
        _NC = _build()
    return _NC


def _prep_weights(weights: np.ndarray) -> np.ndarray:
    # planes 0-2: rows 0-63 = taps (0,m), rows 64-127 = taps (1,m)
    # plane 3: (2,0) | (2,1); plane 4: zero | (2,2)     [5-MM scheme]
    # planes 5-7: zero | (2,kx)                          [6-MM scheme]
    w = np.asarray(weights, dtype=np.float32)
    wt = w.transpose(1, 2, 3, 0)  # [ci, ky, kx, co]
    w8 = np.zeros((128, 8, 128), np.float32)
    w8[0:64, 0:3, :] = wt[:, 0, :, :]
    w8[64:128, 0:3, :] = wt[:, 1, :, :]
    w8[0:64, 3, :] = wt[:, 2, 0, :]
    w8[64:128, 3, :] = wt[:, 2, 1, :]
    w8[64:128, 4, :] = wt[:, 2, 2, :]
    w8[64:128, 5:8, :] = wt[:, 2, :, :]
    return w8.astype(np.float16)


def kernel(input_image: np.ndarray, weights: np.ndarray, _trace: bool = False):
    from concourse.bass_utils import run_bass_kernel_spmd

    nc = _get_nc()
    x16 = np.asarray(input_image).astype(np.float16)
    r = x16.reshape(B_FULL, C_IN, H * W)
    xd = np.zeros((B_FULL, 128, TLEN), np.float16)
    xd[:, 0:64] = r  # A: rows 0..111
    xd[:, 64:128, : TLEN - W] = r[:, :, W:]  # B: rows 1..111, zero pad
    w8 = _prep_weights(weights)
    in_maps = [
        {"x": xd[B_CORE * i : B_CORE * (i + 1)], "w": w8} for i in range(N_CORES)
    ]
    res = run_bass_kernel_spmd(
        nc, in_maps, core_ids=list(range(N_CORES)), trace=_trace
    )
    out = np.concatenate([res.results[i]["y"] for i in range(N_CORES)], axis=0)
    out = out.reshape(B_FULL, C_OUT, OH, OW).astype(np.float32)
    if _trace:
        return out, res
    return out


# revision 35
# speedup vs baseline: 1.0703x; 1.0703x over previous
"""Trainium2 Bass kernel: 3x3 VALID conv2d, stride 1.

Full input [32, 64, 112, 112] f32 + weights [128, 64, 3, 3] f32
-> output [32, 128, 110, 110] f32.

Data-parallel across 8 NeuronCores: 4 images per core.

Per-core formulation: conv as PE matmuls, out = lhsT.T @ rhs with
K (contraction, partitions) = 128 = 64 channels x 2 shifted copies,
M (out partitions) = 128 output channels,
N (moving free dim) = up to 4 input-width rows = 448 (<= 512, one PSUM
bank). The 2 rightmost columns of each 112-wide row are conv garbage;
the PSUM->SBUF copy compacts to the valid 110 columns.

Tap coverage per chunk, two schemes:
  T tile (all chunks): partitions 0-63 = image rows 0..111 (A),
          64-127 = rows 1..111 (B).  Matmuls m=0..2 at column offset
          kx apply tap pairs (0,kx)+(1,kx).
  5-MM scheme (chunks with y0 < Y5=64): U tile: partitions 0-63 =
          rows 2..68 (C), 64-127 = same shifted one column.  m=3
          applies (2,0)+(2,1) in one full-K matmul; m=4 applies (2,2)
          on the hi half only -- 5 matmuls for 9 taps.
  6-MM scheme (y0 >= 64): ky=2 taps via T at row offset +1 with
          zero weights on the A half (planes 5-7) -- 6 matmuls, no U.

U is built on-device by two same-partition contiguous SBUF->SBUF DMAs
per band (lo: A shifted +224 elements; hi: B shifted +113), so HBM
input traffic stays at the single-copy ~12.9 MB/core.  Measured mover
rates force the hybrid coverage: an HWDGE ring moves ~110-125 GB/s
serialized, and every extra consumer slows the others, so U is sized
to what the otherwise-idle scalar ring can sustain alone (~57% of
rows), with image 0's hi half on sync's idle early window.  Each queue
has exactly one role -- tensor: matmuls; vector: PSUM casts; scalar:
U builds; gpsimd: T loads; sync: weights + first bands + output -- so
an unfired semaphore never head-blocks latency-critical work.
(Also measured and rejected: streaming U from HBM saturates the
~358 GB/s HBM interface; full-coverage U overloads the rings and
starves the PE; compute-engine tensor_copy builds crawl at 14-25
G elem/s.)

Moving-N per tap is trimmed (n, n-1, n-2, ...) so no rhs read spills
past input row y0+3 (y0+4 for the 6-MM tail rows): only garbage output
columns lose taps.

Inputs are cast to fp16 on the host (fp32 PE is 4x slower; fp32 PSUM
accumulation keeps rel err ~4e-4).  Output is stored fp16 and cast
back to fp32 on the host, halving output HBM traffic.

A short burst of dummy matmuls on a memset tile runs during the DMA
startup window so the PE HAM clock gate flips to 2.4 GHz by the time
real work arrives.

Schedule: chunks are processed in groups of 8 across the 8 PSUM banks,
weight-plane-major (m outer), so consecutive matmuls hit different
banks (drain overlaps fill).
"""

import numpy as np

B_FULL = 32
N_CORES = 8
B_CORE = B_FULL // N_CORES  # 4 images per core
C_IN = 64
C_OUT = 128
H = W = 112
OH = OW = 110
TLEN = 112 * W  # T plane: rows 0..111 (A) / 1..111 + zero pad (B)
Y5 = 64  # images 0,2,3: 5-MM for y0 >= 64; image 1: all 6-MM.  A U
# band is only safe when needed >=20us after its T band lands (the
# scalar ring + semaphore chain is that slow under load); tail rows of
# images 0/2/3 qualify, image 1's never do.
UBASE = Y5  # U planes cover flat rows 64..109 (image rows 66..111)
ULEN = (OH - Y5) * W  # 46 rows

_NC = None


def _img_chunks():
    # per image: 27 chunks of 4 output rows + 1 of 2 rows = 110
    rows_list = [4] * 27 + [2]
    out = []
    y0 = 0
    for r in rows_list:
        out.append((y0, r))
        y0 += r
    assert y0 == OH
    return out


def _build():
    from contextlib import ExitStack

    import concourse.tile as tile
    from concourse import bacc, mybir

    nc = bacc.Bacc("TRN2", target_bir_lowering=False, debug=False)
    x = nc.dram_tensor(
        "x", [B_CORE, 128, TLEN], mybir.dt.float16, kind="ExternalInput"
    )
    w = nc.dram_tensor("w", [128, 8, 128], mybir.dt.float16, kind="ExternalInput")
    y = nc.dram_tensor(
        "y", [B_CORE, C_OUT, OH * OW], mybir.dt.float16, kind="ExternalOutput"
    )

    chunks = [(b, y0, r) for b in range(B_CORE) for (y0, r) in _img_chunks()]
    assert len(chunks) % 4 == 0
    n_groups = len(chunks) // 4

    with tile.TileContext(nc) as tc, ExitStack() as ctx:
        tpool = ctx.enter_context(tc.tile_pool(name="tp", bufs=B_CORE))
        upool = ctx.enter_context(tc.tile_pool(name="up", bufs=B_CORE - 1))
        wpool = ctx.enter_context(tc.tile_pool(name="wp", bufs=1))
        spool = ctx.enter_context(tc.tile_pool(name="sp", bufs=1))
        opool = ctx.enter_context(tc.tile_pool(name="op", bufs=12))
        ppool = ctx.enter_context(tc.tile_pool(name="pp", bufs=8, space="PSUM"))

        wt = wpool.tile([128, 8 * 128], mybir.dt.float16)
        nc.sync.dma_start(wt[:], w.ap().rearrange("p a b -> p (a b)"))

        # PE warmup: HAM clock gate flips to 2.4 GHz after ~3.4us of
        # sustained activity; burn that in while the first x bands load.
        wu = spool.tile([128, 448], mybir.dt.float16)
        nc.vector.memset(wu[:], 0)
        wu_p = ppool.tile([128, 448], mybir.dt.float32, name="wu_p", tag="pt")
        # wide warmup matmuls: high MAC duty cycle is what flips the HAM
        # clock gate (N=64 warmups never flipped it -- LDW time is idle)
        for _ in range(12):
            nc.tensor.matmul(
                wu_p[0:64, 0:448], wu[:, 0:64], wu[:],
                start=True, stop=True, skip_group_check=True,
            )

        xa = x.ap()
        ya = y.ap()

        # Banded loads so the first chunks start early.  Image 0's first
        # two T bands ride the sync queue (earliest to start); the bulk
        # of T streams on gpsimd/SWDGE.  U band k reads only T band k
        # (U edges = T edges - 2).
        t_tiles = [
            tpool.tile([128, TLEN], mybir.dt.float16, name=f"t{b}", tag="t")
            for b in range(B_CORE)
        ]
        u_tiles = [
            upool.tile([128, ULEN], mybir.dt.float16, name=f"u{b}", tag="u")
            if b != 1 else None
            for b in range(B_CORE)
        ]
        # all T bands ride gpsimd/SWDGE -- the only ring measured at
        # ~200+ GB/s for HBM loads (sync's HWDGE ring crawls at ~80)
        TBs = [[0, 6, 16, 26, 34, 49, 64, 89, 112]] + [[0, 16, 34, 61, 89, 112]] * 3
        UB = [UBASE, 80, 95, OH]

        def t_issue(b, k):
            lo, hi = TBs[b][k], TBs[b][k + 1]
            nc.gpsimd.dma_start(
                t_tiles[b][:, lo * W : hi * W], xa[b][:, lo * W : hi * W]
            )

        def u_build(b, k):
            lo, hi = UB[k], UB[k + 1]
            d0, d1 = (lo - UBASE) * W, (hi - UBASE) * W
            nc.scalar.dma_start(
                u_tiles[b][0:64, d0:d1],
                t_tiles[b][0:64, (lo + 2) * W : (hi + 2) * W],
            )
            nc.scalar.dma_start(
                u_tiles[b][64:128, d0:d1],
                t_tiles[b][64:128, lo * W + 113 : hi * W + 113],
            )

        for b in range(B_CORE):
            for k in range(len(TBs[b]) - 1):
                t_issue(b, k)
        # scalar's queue carries only U builds, so the in-order semaphore
        # waits (T band b,k) head-block nothing; emission order matches
        # T landing order for a natural pipeline
        for b in (0, 2, 3):
            for k in range(len(UB) - 1):
                u_build(b, k)

        def chunk_taps(b, y0, rows):
            n = rows * W
            t, u = t_tiles[b], u_tiles[b]
            five = b != 1 and y0 >= Y5
            if five:
                uo = (y0 - UBASE) * W
                return [
                    (0, t, y0 * W, n),
                    (1, t, y0 * W + 1, n - 1),
                    (2, t, y0 * W + 2, n - 2),
                    (3, u, uo, n - 1),
                    (4, u, uo + 1, n - 2),
                ]
            return [  # 6-matmul scheme, ky=2 via T at +1 row
                (0, t, y0 * W, n),
                (1, t, y0 * W + 1, n - 1),
                (2, t, y0 * W + 2, n - 2),
                (5, t, (y0 + 1) * W, n),
                (6, t, (y0 + 1) * W + 1, n - 1),
                (7, t, (y0 + 1) * W + 2, n - 2),
            ]

        # groups of 4 chunks double-buffer PSUM (4+4 of the 8 banks):
        # group g+1's matmuls overlap group g's casts, so a cast is never
        # on the tensor engine's critical path
        for g in range(n_groups):
            gchunks = chunks[g * 4 : (g + 1) * 4]
            pts = [
                ppool.tile([128, 448], mybir.dt.float32, name="pt", tag="pt")
                for _ in range(4)
            ]
            taps = [chunk_taps(*c) for c in gchunks]
            for m in range(8):
                for j in range(4):
                    for mi, (mm, src, off, nmv) in enumerate(taps[j]):
                        if mm != m:
                            continue
                        nc.tensor.matmul(
                            pts[j][:, 0:nmv],
                            wt[:, m * 128 : (m + 1) * 128],
                            src[:, off : off + nmv],
                            start=(mi == 0),
                            stop=(mi == len(taps[j]) - 1),
                            skip_group_check=True,
                        )
            # compact + store per 2 chunks: copies start draining PSUM as
            # soon as each pair of banks stops; out DMAs alternate between
            # the sync and scalar rings to balance ring load
            for h in range(2):
                pair = gchunks[2 * h : 2 * h + 2]
                total_rows = sum(r for _, _, r in pair)
                ot = opool.tile([128, 8 * OW], mybir.dt.float16, tag="ot")
                off = 0
                for jj, (b, y0, rows) in enumerate(pair):
                    j = 2 * h + jj
                    psrc = pts[j][:].rearrange("p (r c) -> p r c", c=W)[
                        :, 0:rows, 0:OW
                    ]
                    odst = ot[:, off : off + rows * OW].rearrange(
                        "p (r c) -> p r c", c=OW
                    )
                    # all casts on vector: it issues no DMAs, so PSUM
                    # drain is never head-blocked by an unfired T-band
                    # semaphore on a DMA-issuing queue
                    nc.vector.tensor_copy(odst, psrc)
                    off += rows * OW
                b0, y00, _ = pair[0]
                assert all(b == b0 for b, _, _ in pair)
                # later outputs ride the gpsimd ring (the fast SWDGE ring,
                # idle once T has streamed) to kill the output lag/tail
                oeng = nc.gpsimd if g >= n_groups // 2 else nc.sync
                oeng.dma_start(
                    ya[b0][:, y00 * OW : y00 * OW + total_rows * OW],
                    ot[:, 0 : total_rows * OW],
                )

    nc.compile()
    return nc


def _get_nc():
    global _NC
    if _NC is None:
        _NC = _build()
    return _NC


def _prep_weights(weights: np.ndarray) -> np.ndarray:
    # planes 0-2: rows 0-63 = taps (0,m), rows 64-127 = taps (1,m)
    # plane 3: (2,0) | (2,1); plane 4: zero | (2,2)     [5-MM scheme]
    # planes 5-7: zero | (2,kx)                          [6-MM scheme]
    w = np.asarray(weights, dtype=np.float32)
    wt = w.transpose(1, 2, 3, 0)  # [ci, ky, kx, co]
    w8 = np.zeros((128, 8, 128), np.float32)
    w8[0:64, 0:3, :] = wt[:, 0, :, :]
    w8[64:128, 0:3, :] = wt[:, 1, :, :]
    w8[0:64, 3, :] = wt[:, 2, 0, :]
    w8[64:128, 3, :] = wt[:, 2, 1, :]
    w8[64:128, 4, :] = wt[:, 2, 2, :]
    w8[64:128, 5:8, :] = wt[:, 2, :, :]
    return w8.astype(np.float16)


def kernel(input_image: np.ndarray, weights: np.ndarray, _trace: bool = False):
    from concourse.bass_utils import run_bass_kernel_spmd

    nc = _get_nc()
    x16 = np.asarray(input_image).astype(np.float16)
    r = x16.reshape(B_FULL, C_IN, H * W)
    xd = np.zeros((B_FULL, 128, TLEN), np.float16)
    xd[:, 0:64] = r  # A: rows 0..111
    xd[:, 64:128, : TLEN - W] = r[:, :, W:]  # B: rows 1..111, zero pad
    w8 = _prep_weights(weights)
    in_maps = [
        {"x": xd[B_CORE * i : B_CORE * (i + 1)], "w": w8} for i in range(N_CORES)
    ]
    res = run_bass_kernel_spmd(
        nc, in_maps, core_ids=list(range(N_CORES)), trace=_trace
    )
    out = np.concatenate([res.results[i]["y"] for i in range(N_CORES)], axis=0)
    out = out.reshape(B_FULL, C_OUT, OH, OW).astype(np.float32)
    if _trace:
        return out, res
    return out


# revision 36
# speedup vs baseline: 1.0822x; 1.0111x over previous
"""Trainium2 Bass kernel: 3x3 VALID conv2d, stride 1.

Full input [32, 64, 112, 112] f32 + weights [128, 64, 3, 3] f32
-> output [32, 128, 110, 110] f32.

Data-parallel across 8 NeuronCores: 4 images per core.

Per-core formulation: conv as PE matmuls, out = lhsT.T @ rhs with
K (contraction, partitions) = 128 = 64 channels x 2 shifted copies,
M (out partitions) = 128 output channels,
N (moving free dim) = up to 4 input-width rows = 448 (<= 512, one PSUM
bank). The 2 rightmost columns of each 112-wide row are conv garbage;
the PSUM->SBUF copy compacts to the valid 110 columns.

Tap coverage per chunk, two schemes:
  T tile (all chunks): partitions 0-63 = image rows 0..111 (A),
          64-127 = rows 1..111 (B).  Matmuls m=0..2 at column offset
          kx apply tap pairs (0,kx)+(1,kx).
  5-MM scheme (chunks with y0 < Y5=64): U tile: partitions 0-63 =
          rows 2..68 (C), 64-127 = same shifted one column.  m=3
          applies (2,0)+(2,1) in one full-K matmul; m=4 applies (2,2)
          on the hi half only -- 5 matmuls for 9 taps.
  6-MM scheme (y0 >= 64): ky=2 taps via T at row offset +1 with
          zero weights on the A half (planes 5-7) -- 6 matmuls, no U.

U is built on-device by two same-partition contiguous SBUF->SBUF DMAs
per band (lo: A shifted +224 elements; hi: B shifted +113), so HBM
input traffic stays at the single-copy ~12.9 MB/core.  Measured mover
rates force the hybrid coverage: an HWDGE ring moves ~110-125 GB/s
serialized, and every extra consumer slows the others, so U is sized
to what the otherwise-idle scalar ring can sustain alone (~57% of
rows), with image 0's hi half on sync's idle early window.  Each queue
has exactly one role -- tensor: matmuls; vector: PSUM casts; scalar:
U builds; gpsimd: T loads; sync: weights + first bands + output -- so
an unfired semaphore never head-blocks latency-critical work.
(Also measured and rejected: streaming U from HBM saturates the
~358 GB/s HBM interface; full-coverage U overloads the rings and
starves the PE; compute-engine tensor_copy builds crawl at 14-25
G elem/s.)

Moving-N per tap is trimmed (n, n-1, n-2, ...) so no rhs read spills
past input row y0+3 (y0+4 for the 6-MM tail rows): only garbage output
columns lose taps.

Inputs are cast to fp16 on the host (fp32 PE is 4x slower; fp32 PSUM
accumulation keeps rel err ~4e-4).  Output is stored fp16 and cast
back to fp32 on the host, halving output HBM traffic.

A short burst of dummy matmuls on a memset tile runs during the DMA
startup window so the PE HAM clock gate flips to 2.4 GHz by the time
real work arrives.

Schedule: chunks are processed in groups of 8 across the 8 PSUM banks,
weight-plane-major (m outer), so consecutive matmuls hit different
banks (drain overlaps fill).
"""

import numpy as np

B_FULL = 32
N_CORES = 8
B_CORE = B_FULL // N_CORES  # 4 images per core
C_IN = 64
C_OUT = 128
H = W = 112
OH = OW = 110
TLEN = 112 * W  # T plane: rows 0..111 (A) / 1..111 + zero pad (B)
Y5 = 64  # images 0,2,3: 5-MM for y0 >= 64; image 1: all 6-MM.  A U
# band is only safe when needed >=20us after its T band lands (the
# scalar ring + semaphore chain is that slow under load); tail rows of
# images 0/2/3 qualify, image 1's never do.
UBASE = Y5  # U planes cover flat rows 64..109 (image rows 66..111)
ULEN = (OH - Y5) * W  # 46 rows

_NC = None


def _img_chunks():
    # per image: 27 chunks of 4 output rows + 1 of 2 rows = 110
    rows_list = [4] * 27 + [2]
    out = []
    y0 = 0
    for r in rows_list:
        out.append((y0, r))
        y0 += r
    assert y0 == OH
    return out


def _build():
    from contextlib import ExitStack

    import concourse.tile as tile
    from concourse import bacc, mybir

    nc = bacc.Bacc("TRN2", target_bir_lowering=False, debug=False)
    x = nc.dram_tensor(
        "x", [B_CORE, 128, TLEN], mybir.dt.float16, kind="ExternalInput"
    )
    w = nc.dram_tensor("w", [128, 8, 128], mybir.dt.float16, kind="ExternalInput")
    y = nc.dram_tensor(
        "y", [B_CORE, C_OUT, OH * OW], mybir.dt.float16, kind="ExternalOutput"
    )

    chunks = [(b, y0, r) for b in range(B_CORE) for (y0, r) in _img_chunks()]
    assert len(chunks) % 4 == 0
    n_groups = len(chunks) // 4

    with tile.TileContext(nc) as tc, ExitStack() as ctx:
        tpool = ctx.enter_context(tc.tile_pool(name="tp", bufs=B_CORE))
        upool = ctx.enter_context(tc.tile_pool(name="up", bufs=B_CORE - 1))
        wpool = ctx.enter_context(tc.tile_pool(name="wp", bufs=1))
        spool = ctx.enter_context(tc.tile_pool(name="sp", bufs=1))
        opool = ctx.enter_context(tc.tile_pool(name="op", bufs=12))
        ppool = ctx.enter_context(tc.tile_pool(name="pp", bufs=8, space="PSUM"))

        wt = wpool.tile([128, 8 * 128], mybir.dt.float16)
        nc.sync.dma_start(wt[:], w.ap().rearrange("p a b -> p (a b)"))

        # PE warmup: HAM clock gate flips to 2.4 GHz after ~3.4us of
        # sustained activity; burn that in while the first x bands load.
        wu = spool.tile([128, 448], mybir.dt.float16)
        nc.vector.memset(wu[:], 0)
        wu_p = ppool.tile([128, 448], mybir.dt.float32, name="wu_p", tag="pt")
        # wide warmup matmuls: high MAC duty cycle is what flips the HAM
        # clock gate (N=64 warmups never flipped it -- LDW time is idle)
        for _ in range(12):
            nc.tensor.matmul(
                wu_p[0:64, 0:448], wu[:, 0:64], wu[:],
                start=True, stop=True, skip_group_check=True,
            )

        xa = x.ap()
        ya = y.ap()

        # Banded loads so the first chunks start early.  Image 0's first
        # two T bands ride the sync queue (earliest to start); the bulk
        # of T streams on gpsimd/SWDGE.  U band k reads only T band k
        # (U edges = T edges - 2).
        t_tiles = [
            tpool.tile([128, TLEN], mybir.dt.float16, name=f"t{b}", tag="t")
            for b in range(B_CORE)
        ]
        u_tiles = [
            upool.tile([128, ULEN], mybir.dt.float16, name=f"u{b}", tag="u")
            if b != 1 else None
            for b in range(B_CORE)
        ]
        # all T bands ride gpsimd/SWDGE -- the only ring measured at
        # ~200+ GB/s for HBM loads (sync's HWDGE ring crawls at ~80)
        TBs = [[0, 6, 16, 26, 34, 49, 64, 89, 112]] + [
            [0, 16, 34, 48, 61, 75, 89, 112]
        ] * 3
        UB = [UBASE, 80, 95, OH]

        def t_issue(b, k):
            lo, hi = TBs[b][k], TBs[b][k + 1]
            nc.gpsimd.dma_start(
                t_tiles[b][:, lo * W : hi * W], xa[b][:, lo * W : hi * W]
            )

        def u_build(b, k):
            lo, hi = UB[k], UB[k + 1]
            d0, d1 = (lo - UBASE) * W, (hi - UBASE) * W
            nc.scalar.dma_start(
                u_tiles[b][0:64, d0:d1],
                t_tiles[b][0:64, (lo + 2) * W : (hi + 2) * W],
            )
            nc.scalar.dma_start(
                u_tiles[b][64:128, d0:d1],
                t_tiles[b][64:128, lo * W + 113 : hi * W + 113],
            )

        for b in range(B_CORE):
            for k in range(len(TBs[b]) - 1):
                t_issue(b, k)
        # scalar's queue carries only U builds, so the in-order semaphore
        # waits (T band b,k) head-block nothing; emission order matches
        # T landing order for a natural pipeline
        for b in (0, 2, 3):
            for k in range(len(UB) - 1):
                u_build(b, k)

        def chunk_taps(b, y0, rows):
            n = rows * W
            t, u = t_tiles[b], u_tiles[b]
            five = b != 1 and y0 >= Y5
            if five:
                uo = (y0 - UBASE) * W
                return [
                    (0, t, y0 * W, n),
                    (1, t, y0 * W + 1, n - 1),
                    (2, t, y0 * W + 2, n - 2),
                    (3, u, uo, n - 1),
                    (4, u, uo + 1, n - 2),
                ]
            return [  # 6-matmul scheme, ky=2 via T at +1 row
                (0, t, y0 * W, n),
                (1, t, y0 * W + 1, n - 1),
                (2, t, y0 * W + 2, n - 2),
                (5, t, (y0 + 1) * W, n),
                (6, t, (y0 + 1) * W + 1, n - 1),
                (7, t, (y0 + 1) * W + 2, n - 2),
            ]

        # groups of 4 chunks double-buffer PSUM (4+4 of the 8 banks):
        # group g+1's matmuls overlap group g's casts, so a cast is never
        # on the tensor engine's critical path
        for g in range(n_groups):
            gchunks = chunks[g * 4 : (g + 1) * 4]
            pts = [
                ppool.tile([128, 448], mybir.dt.float32, name="pt", tag="pt")
                for _ in range(4)
            ]
            taps = [chunk_taps(*c) for c in gchunks]
            for m in range(8):
                for j in range(4):
                    for mi, (mm, src, off, nmv) in enumerate(taps[j]):
                        if mm != m:
                            continue
                        nc.tensor.matmul(
                            pts[j][:, 0:nmv],
                            wt[:, m * 128 : (m + 1) * 128],
                            src[:, off : off + nmv],
                            start=(mi == 0),
                            stop=(mi == len(taps[j]) - 1),
                            skip_group_check=True,
                        )
            # compact + store per 2 chunks: copies start draining PSUM as
            # soon as each pair of banks stops; out DMAs alternate between
            # the sync and scalar rings to balance ring load
            for h in range(2):
                pair = gchunks[2 * h : 2 * h + 2]
                total_rows = sum(r for _, _, r in pair)
                ot = opool.tile([128, 8 * OW], mybir.dt.float16, tag="ot")
                off = 0
                for jj, (b, y0, rows) in enumerate(pair):
                    j = 2 * h + jj
                    psrc = pts[j][:].rearrange("p (r c) -> p r c", c=W)[
                        :, 0:rows, 0:OW
                    ]
                    odst = ot[:, off : off + rows * OW].rearrange(
                        "p (r c) -> p r c", c=OW
                    )
                    # all casts on vector: it issues no DMAs, so PSUM
                    # drain is never head-blocked by an unfired T-band
                    # semaphore on a DMA-issuing queue
                    nc.vector.tensor_copy(odst, psrc)
                    off += rows * OW
                b0, y00, _ = pair[0]
                assert all(b == b0 for b, _, _ in pair)
                # later outputs ride the gpsimd ring (the fast SWDGE ring,
                # idle once T has streamed) to kill the output lag/tail
                oeng = nc.gpsimd if g >= n_groups // 2 else nc.sync
                oeng.dma_start(
                    ya[b0][:, y00 * OW : y00 * OW + total_rows * OW],
                    ot[:, 0 : total_rows * OW],
                )

    nc.compile()
    return nc


def _get_nc():
    global _NC
    if _NC is None:
        _NC = _build()
    return _NC


def _prep_weights(weights: np.ndarray) -> np.ndarray:
    # planes 0-2: rows 0-63 = taps (0,m), rows 64-127 = taps (1,m)
    # plane 3: (2,0) | (2,1); plane 4: zero | (2,2)     [5-MM scheme]
    # planes 5-7: zero | (2,kx)                          [6-MM scheme]
    w = np.asarray(weights, dtype=np.float32)
    wt = w.transpose(1, 2, 3, 0)  # [ci, ky, kx, co]
    w8 = np.zeros((128, 8, 128), np.float32)
    w8[0:64, 0:3, :] = wt[:, 0, :, :]
    w8[64:128, 0:3, :] = wt[:, 1, :, :]
    w8[0:64, 3, :] = wt[:, 2, 0, :]
    w8[64:128, 3, :] = wt[:, 2, 1, :]
    w8[64:128, 4, :] = wt[:, 2, 2, :]
    w8[64:128, 5:8, :] = wt[:, 2, :, :]
    return w8.astype(np.float16)


def kernel(input_image: np.ndarray, weights: np.ndarray, _trace: bool = False):
    from concourse.bass_utils import run_bass_kernel_spmd

    nc = _get_nc()
    x16 = np.asarray(input_image).astype(np.float16)
    r = x16.reshape(B_FULL, C_IN, H * W)
    xd = np.zeros((B_FULL, 128, TLEN), np.float16)
    xd[:, 0:64] = r  # A: rows 0..111
    xd[:, 64:128, : TLEN - W] = r[:, :, W:]  # B: rows 1..111, zero pad
    w8 = _prep_weights(weights)
    in_maps = [
        {"x": xd[B_CORE * i : B_CORE * (i + 1)], "w": w8} for i in range(N_CORES)
    ]
    res = run_bass_kernel_spmd(
        nc, in_maps, core_ids=list(range(N_CORES)), trace=_trace
    )
    out = np.concatenate([res.results[i]["y"] for i in range(N_CORES)], axis=0)
    out = out.reshape(B_FULL, C_OUT, OH, OW).astype(np.float32)
    if _trace:
        return out, res
    return out


# revision 37
# speedup vs baseline: 1.1022x; 1.0185x over previous
"""Trainium2 Bass kernel: 3x3 VALID conv2d, stride 1.

Full input [32, 64, 112, 112] f32 + weights [128, 64, 3, 3] f32
-> output [32, 128, 110, 110] f32.

Data-parallel across 8 NeuronCores: 4 images per core.

Per-core formulation: conv as PE matmuls, out = lhsT.T @ rhs with
K (contraction, partitions) = 128 = 64 channels x 2 shifted copies,
M (out partitions) = 128 output channels,
N (moving free dim) = up to 4 input-width rows = 448 (<= 512, one PSUM
bank). The 2 rightmost columns of each 112-wide row are conv garbage;
the PSUM->SBUF copy compacts to the valid 110 columns.

Tap coverage per chunk, two schemes:
  T tile (all chunks): partitions 0-63 = image rows 0..111 (A),
          64-127 = rows 1..111 (B).  Matmuls m=0..2 at column offset
          kx apply tap pairs (0,kx)+(1,kx).
  5-MM scheme (chunks with y0 < Y5=64): U tile: partitions 0-63 =
          rows 2..68 (C), 64-127 = same shifted one column.  m=3
          applies (2,0)+(2,1) in one full-K matmul; m=4 applies (2,2)
          on the hi half only -- 5 matmuls for 9 taps.
  6-MM scheme (y0 >= 64): ky=2 taps via T at row offset +1 with
          zero weights on the A half (planes 5-7) -- 6 matmuls, no U.

U is built on-device by two same-partition contiguous SBUF->SBUF DMAs
per band (lo: A shifted +224 elements; hi: B shifted +113), so HBM
input traffic stays at the single-copy ~12.9 MB/core.  Measured mover
rates force the hybrid coverage: an HWDGE ring moves ~110-125 GB/s
serialized, and every extra consumer slows the others, so U is sized
to what the otherwise-idle scalar ring can sustain alone (~57% of
rows), with image 0's hi half on sync's idle early window.  Each queue
has exactly one role -- tensor: matmuls; vector: PSUM casts; scalar:
U builds; gpsimd: T loads; sync: weights + first bands + output -- so
an unfired semaphore never head-blocks latency-critical work.
(Also measured and rejected: streaming U from HBM saturates the
~358 GB/s HBM interface; full-coverage U overloads the rings and
starves the PE; compute-engine tensor_copy builds crawl at 14-25
G elem/s.)

Moving-N per tap is trimmed (n, n-1, n-2, ...) so no rhs read spills
past input row y0+3 (y0+4 for the 6-MM tail rows): only garbage output
columns lose taps.

Inputs are cast to fp16 on the host (fp32 PE is 4x slower; fp32 PSUM
accumulation keeps rel err ~4e-4).  Output is stored fp16 and cast
back to fp32 on the host, halving output HBM traffic.

A short burst of dummy matmuls on a memset tile runs during the DMA
startup window so the PE HAM clock gate flips to 2.4 GHz by the time
real work arrives.

Schedule: chunks are processed in groups of 8 across the 8 PSUM banks,
weight-plane-major (m outer), so consecutive matmuls hit different
banks (drain overlaps fill).
"""

import numpy as np

B_FULL = 32
N_CORES = 8
B_CORE = B_FULL // N_CORES  # 4 images per core
C_IN = 64
C_OUT = 128
H = W = 112
OH = OW = 110
TLEN = 112 * W  # T plane: rows 0..111 (A) / 1..111 + zero pad (B)
Y5 = 64  # images 0,2,3: 5-MM for y0 >= 64; image 1: all 6-MM.  A U
# band is only safe when needed >=20us after its T band lands (the
# scalar ring + semaphore chain is that slow under load); tail rows of
# images 0/2/3 qualify, image 1's never do.
UBASE = Y5  # U planes cover flat rows 64..109 (image rows 66..111)
ULEN = (OH - Y5) * W  # 46 rows

_NC = None


def _img_chunks():
    # per image: 27 chunks of 4 output rows + 1 of 2 rows = 110
    rows_list = [4] * 27 + [2]
    out = []
    y0 = 0
    for r in rows_list:
        out.append((y0, r))
        y0 += r
    assert y0 == OH
    return out


def _build():
    from contextlib import ExitStack

    import concourse.tile as tile
    from concourse import bacc, mybir

    nc = bacc.Bacc("TRN2", target_bir_lowering=False, debug=False)
    x = nc.dram_tensor(
        "x", [B_CORE, 128, TLEN], mybir.dt.float16, kind="ExternalInput"
    )
    w = nc.dram_tensor("w", [128, 8, 128], mybir.dt.float16, kind="ExternalInput")
    y = nc.dram_tensor(
        "y", [B_CORE, C_OUT, OH * OW], mybir.dt.float16, kind="ExternalOutput"
    )

    chunks = [(b, y0, r) for b in range(B_CORE) for (y0, r) in _img_chunks()]
    assert len(chunks) % 4 == 0
    n_groups = len(chunks) // 4

    with tile.TileContext(nc) as tc, ExitStack() as ctx:
        tpool = ctx.enter_context(tc.tile_pool(name="tp", bufs=B_CORE))
        upool = ctx.enter_context(tc.tile_pool(name="up", bufs=B_CORE))
        wpool = ctx.enter_context(tc.tile_pool(name="wp", bufs=1))
        spool = ctx.enter_context(tc.tile_pool(name="sp", bufs=1))
        opool = ctx.enter_context(tc.tile_pool(name="op", bufs=12))
        ppool = ctx.enter_context(tc.tile_pool(name="pp", bufs=8, space="PSUM"))

        wt = wpool.tile([128, 8 * 128], mybir.dt.float16)
        nc.sync.dma_start(wt[:], w.ap().rearrange("p a b -> p (a b)"))

        # PE warmup: HAM clock gate flips to 2.4 GHz after ~3.4us of
        # sustained activity; burn that in while the first x bands load.
        wu = spool.tile([128, 448], mybir.dt.float16)
        nc.vector.memset(wu[:], 0)
        wu_p = ppool.tile([128, 448], mybir.dt.float32, name="wu_p", tag="pt")
        # wide warmup matmuls: high MAC duty cycle is what flips the HAM
        # clock gate (N=64 warmups never flipped it -- LDW time is idle)
        for _ in range(12):
            nc.tensor.matmul(
                wu_p[0:64, 0:448], wu[:, 0:64], wu[:],
                start=True, stop=True, skip_group_check=True,
            )

        xa = x.ap()
        ya = y.ap()

        # Banded loads so the first chunks start early.  Image 0's first
        # two T bands ride the sync queue (earliest to start); the bulk
        # of T streams on gpsimd/SWDGE.  U band k reads only T band k
        # (U edges = T edges - 2).
        t_tiles = [
            tpool.tile([128, TLEN], mybir.dt.float16, name=f"t{b}", tag="t")
            for b in range(B_CORE)
        ]
        u_tiles = [
            upool.tile([128, ULEN], mybir.dt.float16, name=f"u{b}", tag="u")
            for b in range(B_CORE)
        ]
        # all T bands ride gpsimd/SWDGE -- the only ring measured at
        # ~200+ GB/s for HBM loads (sync's HWDGE ring crawls at ~80)
        TBs = [[0, 6, 16, 26, 34, 49, 64, 89, 112]] + [
            [0, 16, 34, 48, 61, 75, 89, 112]
        ] * 3
        UB = [UBASE, 80, 95, OH]
        UB1 = [88, OH]  # image 1: only rows 90..111 (5-MM for y0 >= 88)

        def t_issue(b, k):
            lo, hi = TBs[b][k], TBs[b][k + 1]
            nc.gpsimd.dma_start(
                t_tiles[b][:, lo * W : hi * W], xa[b][:, lo * W : hi * W]
            )

        def u_build(b, k):
            ub = UB1 if b == 1 else UB
            lo, hi = ub[k], ub[k + 1]
            d0, d1 = (lo - UBASE) * W, (hi - UBASE) * W
            nc.scalar.dma_start(
                u_tiles[b][0:64, d0:d1],
                t_tiles[b][0:64, (lo + 2) * W : (hi + 2) * W],
            )
            nc.scalar.dma_start(
                u_tiles[b][64:128, d0:d1],
                t_tiles[b][64:128, lo * W + 113 : hi * W + 113],
            )

        for b in range(B_CORE):
            for k in range(len(TBs[b]) - 1):
                t_issue(b, k)
        # scalar's queue carries only U builds, so the in-order semaphore
        # waits (T band b,k) head-block nothing; emission order matches
        # T landing order for a natural pipeline
        for b in (0, 1, 2, 3):
            ub = UB1 if b == 1 else UB
            for k in range(len(ub) - 1):
                u_build(b, k)

        def chunk_taps(b, y0, rows):
            n = rows * W
            t, u = t_tiles[b], u_tiles[b]
            # image 1's input lands latest relative to consumption, so
            # only its last rows have a safe U-build margin
            five = y0 >= (88 if b == 1 else Y5)
            if five:
                uo = (y0 - UBASE) * W
                return [
                    (0, t, y0 * W, n),
                    (1, t, y0 * W + 1, n - 1),
                    (2, t, y0 * W + 2, n - 2),
                    (3, u, uo, n - 1),
                    (4, u, uo + 1, n - 2),
                ]
            return [  # 6-matmul scheme, ky=2 via T at +1 row
                (0, t, y0 * W, n),
                (1, t, y0 * W + 1, n - 1),
                (2, t, y0 * W + 2, n - 2),
                (5, t, (y0 + 1) * W, n),
                (6, t, (y0 + 1) * W + 1, n - 1),
                (7, t, (y0 + 1) * W + 2, n - 2),
            ]

        # groups of 4 chunks double-buffer PSUM (4+4 of the 8 banks):
        # group g+1's matmuls overlap group g's casts, so a cast is never
        # on the tensor engine's critical path
        for g in range(n_groups):
            gchunks = chunks[g * 4 : (g + 1) * 4]
            pts = [
                ppool.tile([128, 448], mybir.dt.float32, name="pt", tag="pt")
                for _ in range(4)
            ]
            taps = [chunk_taps(*c) for c in gchunks]
            for m in range(8):
                for j in range(4):
                    for mi, (mm, src, off, nmv) in enumerate(taps[j]):
                        if mm != m:
                            continue
                        nc.tensor.matmul(
                            pts[j][:, 0:nmv],
                            wt[:, m * 128 : (m + 1) * 128],
                            src[:, off : off + nmv],
                            start=(mi == 0),
                            stop=(mi == len(taps[j]) - 1),
                            skip_group_check=True,
                        )
            # compact + store per 2 chunks: copies start draining PSUM as
            # soon as each pair of banks stops; out DMAs alternate between
            # the sync and scalar rings to balance ring load
            for h in range(2):
                pair = gchunks[2 * h : 2 * h + 2]
                total_rows = sum(r for _, _, r in pair)
                ot = opool.tile([128, 8 * OW], mybir.dt.float16, tag="ot")
                off = 0
                for jj, (b, y0, rows) in enumerate(pair):
                    j = 2 * h + jj
                    psrc = pts[j][:].rearrange("p (r c) -> p r c", c=W)[
                        :, 0:rows, 0:OW
                    ]
                    odst = ot[:, off : off + rows * OW].rearrange(
                        "p (r c) -> p r c", c=OW
                    )
                    # casts ride vector (it issues no DMAs, so PSUM drain
                    # is never head-blocked by an unfired semaphore on a
                    # DMA-issuing queue); scalar, idle by the tail, takes
                    # half of the last groups' casts to shorten the drain
                    if g >= n_groups - 2 and jj == 1:
                        nc.scalar.copy(odst, psrc)
                    else:
                        nc.vector.tensor_copy(odst, psrc)
                    off += rows * OW
                b0, y00, _ = pair[0]
                assert all(b == b0 for b, _, _ in pair)
                # later outputs ride the gpsimd ring (the fast SWDGE ring,
                # idle once T has streamed) to kill the output lag/tail
                oeng = nc.gpsimd if g >= n_groups // 2 else nc.sync
                oeng.dma_start(
                    ya[b0][:, y00 * OW : y00 * OW + total_rows * OW],
                    ot[:, 0 : total_rows * OW],
                )

    nc.compile()
    return nc


def _get_nc():
    global _NC
    if _NC is None:
        _NC = _build()
    return _NC


def _prep_weights(weights: np.ndarray) -> np.ndarray:
    # planes 0-2: rows 0-63 = taps (0,m), rows 64-127 = taps (1,m)
    # plane 3: (2,0) | (2,1); plane 4: zero | (2,2)     [5-MM scheme]
    # planes 5-7: zero | (2,kx)                          [6-MM scheme]
    w = np.asarray(weights, dtype=np.float32)
    wt = w.transpose(1, 2, 3, 0)  # [ci, ky, kx, co]
    w8 = np.zeros((128, 8, 128), np.float32)
    w8[0:64, 0:3, :] = wt[:, 0, :, :]
    w8[64:128, 0:3, :] = wt[:, 1, :, :]
    w8[0:64, 3, :] = wt[:, 2, 0, :]
    w8[64:128, 3, :] = wt[:, 2, 1, :]
    w8[64:128, 4, :] = wt[:, 2, 2, :]
    w8[64:128, 5:8, :] = wt[:, 2, :, :]
    return w8.astype(np.float16)


def kernel(input_image: np.ndarray, weights: np.ndarray, _trace: bool = False):
    from concourse.bass_utils import run_bass_kernel_spmd

    nc = _get_nc()
    x16 = np.asarray(input_image).astype(np.float16)
    r = x16.reshape(B_FULL, C_IN, H * W)
    xd = np.zeros((B_FULL, 128, TLEN), np.float16)
    xd[:, 0:64] = r  # A: rows 0..111
    xd[:, 64:128, : TLEN - W] = r[:, :, W:]  # B: rows 1..111, zero pad
    w8 = _prep_weights(weights)
    in_maps = [
        {"x": xd[B_CORE * i : B_CORE * (i + 1)], "w": w8} for i in range(N_CORES)
    ]
    res = run_bass_kernel_spmd(
        nc, in_maps, core_ids=list(range(N_CORES)), trace=_trace
    )
    out = np.concatenate([res.results[i]["y"] for i in range(N_CORES)], axis=0)
    out = out.reshape(B_FULL, C_OUT, OH, OW).astype(np.float32)
    if _trace:
        return out, res
    return out


# revision 38
# speedup vs baseline: 1.1088x; 1.0060x over previous
"""Trainium2 Bass kernel: 3x3 VALID conv2d, stride 1.

Full input [32, 64, 112, 112] f32 + weights [128, 64, 3, 3] f32
-> output [32, 128, 110, 110] f32.

Data-parallel across 8 NeuronCores: 4 images per core.

Per-core formulation: conv as PE matmuls, out = lhsT.T @ rhs with
K (contraction, partitions) = 128 = 64 channels x 2 shifted copies,
M (out partitions) = 128 output channels,
N (moving free dim) = up to 4 input-width rows = 448 (<= 512, one PSUM
bank). The 2 rightmost columns of each 112-wide row are conv garbage;
the PSUM->SBUF copy compacts to the valid 110 columns.

Tap coverage per chunk, two schemes:
  T tile (all chunks): partitions 0-63 = image rows 0..111 (A),
          64-127 = rows 1..111 (B).  Matmuls m=0..2 at column offset
          kx apply tap pairs (0,kx)+(1,kx).
  5-MM scheme (chunks with y0 < Y5=64): U tile: partitions 0-63 =
          rows 2..68 (C), 64-127 = same shifted one column.  m=3
          applies (2,0)+(2,1) in one full-K matmul; m=4 applies (2,2)
          on the hi half only -- 5 matmuls for 9 taps.
  6-MM scheme (y0 >= 64): ky=2 taps via T at row offset +1 with
          zero weights on the A half (planes 5-7) -- 6 matmuls, no U.

U is built on-device by two same-partition contiguous SBUF->SBUF DMAs
per band (lo: A shifted +224 elements; hi: B shifted +113), so HBM
input traffic stays at the single-copy ~12.9 MB/core.  Measured mover
rates force the hybrid coverage: an HWDGE ring moves ~110-125 GB/s
serialized, and every extra consumer slows the others, so U is sized
to what the otherwise-idle scalar ring can sustain alone (~57% of
rows), with image 0's hi half on sync's idle early window.  Each queue
has exactly one role -- tensor: matmuls; vector: PSUM casts; scalar:
U builds; gpsimd: T loads; sync: weights + first bands + output -- so
an unfired semaphore never head-blocks latency-critical work.
(Also measured and rejected: streaming U from HBM saturates the
~358 GB/s HBM interface; full-coverage U overloads the rings and
starves the PE; compute-engine tensor_copy builds crawl at 14-25
G elem/s.)

Moving-N per tap is trimmed (n, n-1, n-2, ...) so no rhs read spills
past input row y0+3 (y0+4 for the 6-MM tail rows): only garbage output
columns lose taps.

Inputs are cast to fp16 on the host (fp32 PE is 4x slower; fp32 PSUM
accumulation keeps rel err ~4e-4).  Output is stored fp16 and cast
back to fp32 on the host, halving output HBM traffic.

A short burst of dummy matmuls on a memset tile runs during the DMA
startup window so the PE HAM clock gate flips to 2.4 GHz by the time
real work arrives.

Schedule: chunks are processed in groups of 8 across the 8 PSUM banks,
weight-plane-major (m outer), so consecutive matmuls hit different
banks (drain overlaps fill).
"""

import numpy as np

B_FULL = 32
N_CORES = 8
B_CORE = B_FULL // N_CORES  # 4 images per core
C_IN = 64
C_OUT = 128
H = W = 112
OH = OW = 110
TLEN = 112 * W  # T plane: rows 0..111 (A) / 1..111 + zero pad (B)
# 5-MM start row per image: a U band is only safe when needed >=15us
# after its T band lands (the scalar ring + semaphore chain is that
# slow under load).  Later images have later need-times, so coverage
# grows: image 0 tail-only, image 1 last rows, images 2-3 from y0=32.
Y5S = [64, 88, 32, 32]
UBASE = 32  # all U planes share the rows-34..111 layout
ULEN = (OH - UBASE) * W

_NC = None


def _img_chunks():
    # per image: 27 chunks of 4 output rows + 1 of 2 rows = 110
    rows_list = [4] * 27 + [2]
    out = []
    y0 = 0
    for r in rows_list:
        out.append((y0, r))
        y0 += r
    assert y0 == OH
    return out


def _build():
    from contextlib import ExitStack

    import concourse.tile as tile
    from concourse import bacc, mybir

    nc = bacc.Bacc("TRN2", target_bir_lowering=False, debug=False)
    x = nc.dram_tensor(
        "x", [B_CORE, 128, TLEN], mybir.dt.float16, kind="ExternalInput"
    )
    w = nc.dram_tensor("w", [128, 8, 128], mybir.dt.float16, kind="ExternalInput")
    y = nc.dram_tensor(
        "y", [B_CORE, C_OUT, OH * OW], mybir.dt.float16, kind="ExternalOutput"
    )

    chunks = [(b, y0, r) for b in range(B_CORE) for (y0, r) in _img_chunks()]
    assert len(chunks) % 4 == 0
    n_groups = len(chunks) // 4

    with tile.TileContext(nc) as tc, ExitStack() as ctx:
        tpool = ctx.enter_context(tc.tile_pool(name="tp", bufs=B_CORE))
        upool = ctx.enter_context(tc.tile_pool(name="up", bufs=B_CORE))
        wpool = ctx.enter_context(tc.tile_pool(name="wp", bufs=1))
        spool = ctx.enter_context(tc.tile_pool(name="sp", bufs=1))
        opool = ctx.enter_context(tc.tile_pool(name="op", bufs=12))
        ppool = ctx.enter_context(tc.tile_pool(name="pp", bufs=8, space="PSUM"))

        wt = wpool.tile([128, 8 * 128], mybir.dt.float16)
        nc.sync.dma_start(wt[:], w.ap().rearrange("p a b -> p (a b)"))

        # PE warmup: HAM clock gate flips to 2.4 GHz after ~3.4us of
        # sustained activity; burn that in while the first x bands load.
        wu = spool.tile([128, 448], mybir.dt.float16)
        nc.vector.memset(wu[:], 0)
        wu_p = ppool.tile([128, 448], mybir.dt.float32, name="wu_p", tag="pt")
        # wide warmup matmuls: high MAC duty cycle is what flips the HAM
        # clock gate (N=64 warmups never flipped it -- LDW time is idle)
        for _ in range(12):
            nc.tensor.matmul(
                wu_p[0:64, 0:448], wu[:, 0:64], wu[:],
                start=True, stop=True, skip_group_check=True,
            )

        xa = x.ap()
        ya = y.ap()

        # Banded loads so the first chunks start early.  Image 0's first
        # two T bands ride the sync queue (earliest to start); the bulk
        # of T streams on gpsimd/SWDGE.  U band k reads only T band k
        # (U edges = T edges - 2).
        t_tiles = [
            tpool.tile([128, TLEN], mybir.dt.float16, name=f"t{b}", tag="t")
            for b in range(B_CORE)
        ]
        u_tiles = [
            upool.tile([128, ULEN], mybir.dt.float16, name=f"u{b}", tag="u")
            for b in range(B_CORE)
        ]
        # all T bands ride gpsimd/SWDGE -- the only ring measured at
        # ~200+ GB/s for HBM loads (sync's HWDGE ring crawls at ~80)
        TBs = [[0, 6, 16, 26, 34, 49, 64, 89, 112]] + [
            [0, 16, 34, 48, 61, 75, 89, 112]
        ] * 3
        UBS = {
            0: [64, 80, 95, OH],
            1: [88, OH],
            2: [32, 48, 64, 80, 95, OH],
            3: [32, 48, 64, 80, 95, OH],
        }

        def t_issue(b, k):
            lo, hi = TBs[b][k], TBs[b][k + 1]
            nc.gpsimd.dma_start(
                t_tiles[b][:, lo * W : hi * W], xa[b][:, lo * W : hi * W]
            )

        def u_build(b, k):
            ub = UBS[b]
            lo, hi = ub[k], ub[k + 1]
            d0, d1 = (lo - UBASE) * W, (hi - UBASE) * W
            nc.scalar.dma_start(
                u_tiles[b][0:64, d0:d1],
                t_tiles[b][0:64, (lo + 2) * W : (hi + 2) * W],
            )
            nc.scalar.dma_start(
                u_tiles[b][64:128, d0:d1],
                t_tiles[b][64:128, lo * W + 113 : hi * W + 113],
            )

        for b in range(B_CORE):
            for k in range(len(TBs[b]) - 1):
                t_issue(b, k)
        # scalar's queue carries only U builds, so the in-order semaphore
        # waits (T band b,k) head-block nothing; emission order matches
        # T landing order for a natural pipeline
        for b in (0, 1, 2, 3):
            for k in range(len(UBS[b]) - 1):
                u_build(b, k)

        def chunk_taps(b, y0, rows):
            n = rows * W
            t, u = t_tiles[b], u_tiles[b]
            five = y0 >= Y5S[b]
            if five:
                uo = (y0 - UBASE) * W
                return [
                    (0, t, y0 * W, n),
                    (1, t, y0 * W + 1, n - 1),
                    (2, t, y0 * W + 2, n - 2),
                    (3, u, uo, n - 1),
                    (4, u, uo + 1, n - 2),
                ]
            return [  # 6-matmul scheme, ky=2 via T at +1 row
                (0, t, y0 * W, n),
                (1, t, y0 * W + 1, n - 1),
                (2, t, y0 * W + 2, n - 2),
                (5, t, (y0 + 1) * W, n),
                (6, t, (y0 + 1) * W + 1, n - 1),
                (7, t, (y0 + 1) * W + 2, n - 2),
            ]

        # groups of 4 chunks double-buffer PSUM (4+4 of the 8 banks):
        # group g+1's matmuls overlap group g's casts, so a cast is never
        # on the tensor engine's critical path
        for g in range(n_groups):
            gchunks = chunks[g * 4 : (g + 1) * 4]
            pts = [
                ppool.tile([128, 448], mybir.dt.float32, name="pt", tag="pt")
                for _ in range(4)
            ]
            taps = [chunk_taps(*c) for c in gchunks]
            for m in range(8):
                for j in range(4):
                    for mi, (mm, src, off, nmv) in enumerate(taps[j]):
                        if mm != m:
                            continue
                        nc.tensor.matmul(
                            pts[j][:, 0:nmv],
                            wt[:, m * 128 : (m + 1) * 128],
                            src[:, off : off + nmv],
                            start=(mi == 0),
                            stop=(mi == len(taps[j]) - 1),
                            skip_group_check=True,
                        )
            # compact + store per 2 chunks: copies start draining PSUM as
            # soon as each pair of banks stops; out DMAs alternate between
            # the sync and scalar rings to balance ring load
            for h in range(2):
                pair = gchunks[2 * h : 2 * h + 2]
                total_rows = sum(r for _, _, r in pair)
                ot = opool.tile([128, 8 * OW], mybir.dt.float16, tag="ot")
                off = 0
                for jj, (b, y0, rows) in enumerate(pair):
                    j = 2 * h + jj
                    psrc = pts[j][:].rearrange("p (r c) -> p r c", c=W)[
                        :, 0:rows, 0:OW
                    ]
                    odst = ot[:, off : off + rows * OW].rearrange(
                        "p (r c) -> p r c", c=OW
                    )
                    # casts ride vector (it issues no DMAs, so PSUM drain
                    # is never head-blocked by an unfired semaphore on a
                    # DMA-issuing queue); scalar, idle by the tail, takes
                    # half of the last groups' casts to shorten the drain
                    if g >= n_groups - 2 and jj == 1:
                        nc.scalar.copy(odst, psrc)
                    else:
                        nc.vector.tensor_copy(odst, psrc)
                    off += rows * OW
                b0, y00, _ = pair[0]
                assert all(b == b0 for b, _, _ in pair)
                # later outputs ride the gpsimd ring (the fast SWDGE ring,
                # idle once T has streamed) to kill the output lag/tail
                oeng = nc.gpsimd if g >= n_groups // 2 else nc.sync
                oeng.dma_start(
                    ya[b0][:, y00 * OW : y00 * OW + total_rows * OW],
                    ot[:, 0 : total_rows * OW],
                )

    nc.compile()
    return nc


def _get_nc():
    global _NC
    if _NC is None:
        _NC = _build()
    return _NC


def _prep_weights(weights: np.ndarray) -> np.ndarray:
    # planes 0-2: rows 0-63 = taps (0,m), rows 64-127 = taps (1,m)
    # plane 3: (2,0) | (2,1); plane 4: zero | (2,2)     [5-MM scheme]
    # planes 5-7: zero | (2,kx)                          [6-MM scheme]
    w = np.asarray(weights, dtype=np.float32)
    wt = w.transpose(1, 2, 3, 0)  # [ci, ky, kx, co]
    w8 = np.zeros((128, 8, 128), np.float32)
    w8[0:64, 0:3, :] = wt[:, 0, :, :]
    w8[64:128, 0:3, :] = wt[:, 1, :, :]
    w8[0:64, 3, :] = wt[:, 2, 0, :]
    w8[64:128, 3, :] = wt[:, 2, 1, :]
    w8[64:128, 4, :] = wt[:, 2, 2, :]
    w8[64:128, 5:8, :] = wt[:, 2, :, :]
    return w8.astype(np.float16)


def kernel(input_image: np.ndarray, weights: np.ndarray, _trace: bool = False):
    from concourse.bass_utils import run_bass_kernel_spmd

    nc = _get_nc()
    x16 = np.asarray(input_image).astype(np.float16)
    r = x16.reshape(B_FULL, C_IN, H * W)
    xd = np.zeros((B_FULL, 128, TLEN), np.float16)
    xd[:, 0:64] = r  # A: rows 0..111
    xd[:, 64:128, : TLEN - W] = r[:, :, W:]  # B: rows 1..111, zero pad
    w8 = _prep_weights(weights)
    in_maps = [
        {"x": xd[B_CORE * i : B_CORE * (i + 1)], "w": w8} for i in range(N_CORES)
    ]
    res = run_bass_kernel_spmd(
        nc, in_maps, core_ids=list(range(N_CORES)), trace=_trace
    )
    out = np.concatenate([res.results[i]["y"] for i in range(N_CORES)], axis=0)
    out = out.reshape(B_FULL, C_OUT, OH, OW).astype(np.float32)
    if _trace:
        return out, res
    return out
